# revision 7
# baseline (speedup 1.0000x reference)
"""VQ codebook layer (EuclideanSnapFunction) on 8 Trainium2 NeuronCores.

Math: for each token x_t (768-d), scores over 8192 codes:
    s[t,n] = x_t . c_n - 0.5*||c_n||^2
which orders identically to -||x_t - c_n||^2 (the per-token ||x_t||^2 shift
does not change per-row order). Top-8 codes per token (descending score,
ties -> lower index, matching jax.lax.top_k), then outputs = mean of the 8
gathered codebook rows.

Sharding: data-parallel over tokens: 8192 tokens -> 8 cores x 1024 tokens,
codebook replicated.

Precision: scores are computed with a 3-term fp16 split matmul
    x.c ~= xh.ch + xh.cl + xl.ch   (xh=f16(x), xl=f16(x-xh), etc)
which carries ~21 significand bits through the fp32 PSUM accumulator;
measured |err| ~3e-5 vs fp64 — at the same level as a plain fp32 matmul —
while streaming at the full 1 cycle/row PE rate (fp32 matmul is 4 cyc/row).
The -0.5*||c||^2 bias is folded into the same PSUM accumulation as a K=3
fp16 matmul of a ones-vector against a 3-term fp16 split of the bias row.

Per core: 16 code-chunks of 512 stream through PSUM; per 128-token tile a
chunk-local top-8 (DVE max8/max8-index) produces 16x8 candidates; a final
max8 over the 128 candidate values + one-hot index extraction yields the
global top-8 ids. Codebook rows are gathered with indirect DMA and averaged.
"""
import numpy as np

try:
    import concourse  # noqa: F401
except ImportError:
    import sys
    sys.path.insert(0, "/opt/trn_rl_repo")

N_CORES = 8
P = 128
D = 768
KD = D // P            # 6 contraction chunks of 128
NCODES = 8192
TOK = 1024             # tokens per core
M_TILES = TOK // P     # 8 token tiles per core
NB = 512               # codes per PSUM chunk
N_CH = NCODES // NB    # 16 chunks
K = 8                  # top-k

_CACHE = {}


def _build():
    import concourse.tile as tile
    from concourse import bacc, mybir

    nc = bacc.Bacc("TRN2", target_bir_lowering=False, debug=False,
                   num_devices=N_CORES)
    f16 = mybir.dt.float16
    f32 = mybir.dt.float32
    u32 = mybir.dt.uint32
    i32 = mybir.dt.int32

    xh_in = nc.dram_tensor("xh", [KD, P, TOK], f16, kind="ExternalInput")
    xl_in = nc.dram_tensor("xl", [KD, P, TOK], f16, kind="ExternalInput")
    ch_in = nc.dram_tensor("ch", [KD, P, NCODES], f16, kind="ExternalInput")
    cl_in = nc.dram_tensor("cl", [KD, P, NCODES], f16, kind="ExternalInput")
    b3_in = nc.dram_tensor("b3", [P, NCODES], f16, kind="ExternalInput")
    cb_in = nc.dram_tensor("cb", [NCODES, D], f32, kind="ExternalInput")
    iota_in = nc.dram_tensor("iota", [P, P], f32, kind="ExternalInput")
    offs_in = nc.dram_tensor("offs", [P, N_CH * K], f32, kind="ExternalInput")

    out_o = nc.dram_tensor("out", [TOK, D], f32, kind="ExternalOutput")
    ids_o = nc.dram_tensor("ids", [TOK, K], i32, kind="ExternalOutput")

    with tile.TileContext(nc) as tc:
        with (
            tc.tile_pool(name="const", bufs=1) as cpool,
            tc.tile_pool(name="cstream", bufs=3) as cspool,
            tc.tile_pool(name="sc", bufs=6) as spool,
            tc.tile_pool(name="merge", bufs=2) as mpool,
            tc.tile_pool(name="gath", bufs=2) as gpool,
            tc.tile_pool(name="outp", bufs=2) as opool,
            tc.tile_pool(name="psum", bufs=6, space="PSUM") as ppool,
        ):
            # --- resident constants ---
            xh = cpool.tile([P, KD * TOK], f16)
            xl = cpool.tile([P, KD * TOK], f16)
            for k in range(KD):
                nc.sync.dma_start(xh[:, k * TOK:(k + 1) * TOK], xh_in[k])
                nc.sync.dma_start(xl[:, k * TOK:(k + 1) * TOK], xl_in[k])
            b3 = cpool.tile([P, NCODES], f16)
            nc.sync.dma_start(b3[:], b3_in[:])
            ones3 = cpool.tile([P, P], f16)
            nc.vector.memset(ones3[:], 1.0)
            iota = cpool.tile([P, P], f32)
            nc.sync.dma_start(iota[:], iota_in[:])
            offs = cpool.tile([P, N_CH * K], f32)
            nc.sync.dma_start(offs[:], offs_in[:])
            # candidate values / local idx for all token tiles
            cv = cpool.tile([P, M_TILES * N_CH * K], f32)
            ci = cpool.tile([P, M_TILES * N_CH * K], u32)

            # --- score + chunk-local top-8 ---
            for n in range(N_CH):
                ns = slice(n * NB, (n + 1) * NB)
                cht = cspool.tile([P, KD * NB], f16, tag="cht")
                clt = cspool.tile([P, KD * NB], f16, tag="clt")
                for k in range(KD):
                    nc.sync.dma_start(cht[:, k * NB:(k + 1) * NB], ch_in[k, :, ns])
                    nc.sync.dma_start(clt[:, k * NB:(k + 1) * NB], cl_in[k, :, ns])
                for m in range(M_TILES):
                    ps = ppool.tile([P, NB], f32)
                    nc.tensor.matmul(ps[:], lhsT=ones3[:], rhs=b3[:, ns],
                                     start=True, stop=False)
                    i = 0
                    for a, b in ((xh, cht), (xh, clt), (xl, cht)):
                        for k in range(KD):
                            nc.tensor.matmul(
                                ps[:],
                                lhsT=a[:, k * TOK + m * P: k * TOK + (m + 1) * P],
                                rhs=b[:, k * NB:(k + 1) * NB],
                                start=False, stop=(i == 3 * KD - 1),
                            )
                            i += 1
                    sc = spool.tile([P, NB], f32, tag="sc")
                    nc.scalar.copy(sc[:], ps[:])
                    cvs = cv[:, (m * N_CH + n) * K: (m * N_CH + n + 1) * K]
                    cis = ci[:, (m * N_CH + n) * K: (m * N_CH + n + 1) * K]
                    nc.vector.max(cvs, sc[:])
                    nc.vector.max_index(cis, cvs, sc[:])

            # --- merge + gather + mean per token tile ---
            inv_k = 1.0 / K
            for m in range(M_TILES):
                cvm = cv[:, m * N_CH * K: (m + 1) * N_CH * K]
                cim = ci[:, m * N_CH * K: (m + 1) * N_CH * K]
                cif = mpool.tile([P, N_CH * K], f32, tag="cif")
                nc.vector.tensor_copy(cif[:], cim)
                gid = mpool.tile([P, N_CH * K], f32, tag="gid")
                nc.vector.tensor_add(gid[:], cif[:], offs[:])
                v8 = mpool.tile([P, K], f32, tag="v8")
                nc.vector.max(v8[:], cvm)
                pos8 = mpool.tile([P, K], u32, tag="pos8")
                nc.vector.max_index(pos8[:], v8[:], cvm)
                pos8f = mpool.tile([P, K], f32, tag="pos8f")
                nc.vector.tensor_copy(pos8f[:], pos8[:])
                idsf = mpool.tile([P, K], f32, tag="idsf")
                from concourse import mybir as mb
                for k in range(K):
                    sel = mpool.tile([P, N_CH * K], f32, tag="sel")
                    nc.vector.scalar_tensor_tensor(
                        sel[:], iota[:], pos8f[:, k:k + 1], gid[:],
                        op0=mb.AluOpType.is_equal, op1=mb.AluOpType.mult,
                    )
                    nc.vector.tensor_reduce(
                        idsf[:, k:k + 1], sel[:],
                        axis=mb.AxisListType.X, op=mb.AluOpType.add,
                    )
                ids32 = mpool.tile([P, K], i32, tag="ids32")
                nc.vector.tensor_copy(ids32[:], idsf[:])
                nc.sync.dma_start(ids_o[m * P:(m + 1) * P, :], ids32[:])

                import concourse.bass as bass
                g = gpool.tile([P, K, D], f32, tag="g")
                for k in range(K):
                    nc.gpsimd.indirect_dma_start(
                        out=g[:, k, :], out_offset=None,
                        in_=cb_in[:],
                        in_offset=bass.IndirectOffsetOnAxis(
                            ap=ids32[:, k:k + 1], axis=0),
                    )
                acc = opool.tile([P, D], f32, tag="acc")
                nc.vector.tensor_add(acc[:], g[:, 0, :], g[:, 1, :])
                for k in range(2, K):
                    nc.vector.tensor_add(acc[:], acc[:], g[:, k, :])
                outt = opool.tile([P, D], f32, tag="outt")
                nc.scalar.mul(outt[:], acc[:], inv_k)
                nc.sync.dma_start(out_o[m * P:(m + 1) * P, :], outt[:])

    nc.compile()
    return nc


def _get_nc():
    if "nc" not in _CACHE:
        _CACHE["nc"] = _build()
    return _CACHE["nc"]


def _split_f16(a32):
    """3-term fp16 split helper: returns (hi, lo) with hi+lo ~ a32 (f32)."""
    hi = a32.astype(np.float16)
    lo = (a32 - hi.astype(np.float32)).astype(np.float16)
    return hi, lo


def _prep_in_maps(x, codebook):
    B, S, _ = x.shape
    xf = np.ascontiguousarray(x.reshape(B * S, D))
    cb = np.ascontiguousarray(codebook.astype(np.float32, copy=False))

    # codebook-side prep (shared by all cores)
    cT = cb.T.astype(np.float32)                      # [768, 8192]
    ch, cl = _split_f16(cT)
    ch = np.ascontiguousarray(ch.reshape(KD, P, NCODES))
    cl = np.ascontiguousarray(cl.reshape(KD, P, NCODES))
    bias = (-0.5 * (cb.astype(np.float64) ** 2).sum(1)).astype(np.float32)
    b0 = bias.astype(np.float16)
    r = bias - b0.astype(np.float32)
    b1 = r.astype(np.float16)
    b2 = (r - b1.astype(np.float32)).astype(np.float16)
    b3 = np.zeros((P, NCODES), np.float16)
    b3[0], b3[1], b3[2] = b0, b1, b2  # zero-padded to K=128 rows
    iota = np.broadcast_to(np.arange(N_CH * K, dtype=np.float32), (P, N_CH * K))
    iota = np.ascontiguousarray(iota)
    offs = np.broadcast_to(
        (np.arange(N_CH * K, dtype=np.float32) // K).astype(np.float32) * NB,
        (P, N_CH * K))
    offs = np.ascontiguousarray(offs)

    in_maps = []
    for c in range(N_CORES):
        xs = xf[c * TOK:(c + 1) * TOK]                # [1024, 768]
        xT = np.ascontiguousarray(xs.T)               # [768, 1024]
        xh, xl = _split_f16(xT)
        in_maps.append({
            "xh": np.ascontiguousarray(xh.reshape(KD, P, TOK)),
            "xl": np.ascontiguousarray(xl.reshape(KD, P, TOK)),
            "ch": ch, "cl": cl, "b3": b3, "cb": cb,
            "iota": iota, "offs": offs,
        })
    return in_maps


def kernel(x, codebook):
    from concourse.bass_utils import run_bass_kernel_spmd

    nc = _get_nc()
    B, S, _ = x.shape
    in_maps = _prep_in_maps(x, codebook)

    last_err = None
    for _attempt in range(3):
        try:
            res = run_bass_kernel_spmd(nc, in_maps, list(range(N_CORES)))
            break
        except Exception as e:  # device wedge — retry
            last_err = e
    else:
        raise last_err
    outs = np.concatenate([res.results[c]["out"] for c in range(N_CORES)])
    ids = np.concatenate([res.results[c]["ids"] for c in range(N_CORES)])
    return outs.reshape(B, S, D), ids.reshape(B, S, K).astype(np.int32)


# revision 9
# speedup vs baseline: 1.0201x; 1.0201x over previous
"""VQ codebook layer (EuclideanSnapFunction) on 8 Trainium2 NeuronCores.

Math: for each token x_t (768-d), scores over 8192 codes:
    s[t,n] = x_t . c_n - 0.5*||c_n||^2
which orders identically to -||x_t - c_n||^2 (the per-token ||x_t||^2 shift
does not change per-row order). Top-8 codes per token (descending score,
ties -> lower index, matching jax.lax.top_k), then outputs = mean of the 8
gathered codebook rows.

Sharding: data-parallel over tokens: 8192 tokens -> 8 cores x 1024 tokens,
codebook replicated.

Precision: scores are computed with a 3-term fp16 split matmul
    x.c ~= xh.ch + xh.cl + xl.ch   (xh=f16(x), xl=f16(x-xh), etc)
which carries ~21 significand bits through the fp32 PSUM accumulator;
measured |err| ~3e-5 vs fp64 — at the same level as a plain fp32 matmul —
while streaming at the full 1 cycle/row PE rate (fp32 matmul is 4 cyc/row).
The -0.5*||c||^2 bias is folded into the same PSUM accumulation as a K=3
fp16 matmul of a ones-vector against a 3-term fp16 split of the bias row.

Per core: 16 code-chunks of 512 stream through PSUM; per 128-token tile a
chunk-local top-8 (DVE max8/max8-index) produces 16x8 candidates; a final
max8 over the 128 candidate values + one-hot index extraction yields the
global top-8 ids. Codebook rows are gathered with indirect DMA and averaged.
"""
import numpy as np

try:
    import concourse  # noqa: F401
except ImportError:
    import sys
    sys.path.insert(0, "/opt/trn_rl_repo")

N_CORES = 8
P = 128
D = 768
KD = D // P            # 6 contraction chunks of 128
NCODES = 8192
TOK = 1024             # tokens per core
M_TILES = TOK // P     # 8 token tiles per core
NB = 512               # codes per PSUM chunk
N_CH = NCODES // NB    # 16 chunks
K = 8                  # top-k

_CACHE = {}


def _build():
    import concourse.tile as tile
    from concourse import bacc, mybir

    nc = bacc.Bacc("TRN2", target_bir_lowering=False, debug=False,
                   num_devices=N_CORES)
    f16 = mybir.dt.float16
    f32 = mybir.dt.float32
    u32 = mybir.dt.uint32
    i32 = mybir.dt.int32

    xh_in = nc.dram_tensor("xh", [KD, P, TOK], f16, kind="ExternalInput")
    xl_in = nc.dram_tensor("xl", [KD, P, TOK], f16, kind="ExternalInput")
    ch_in = nc.dram_tensor("ch", [KD, P, NCODES], f16, kind="ExternalInput")
    cl_in = nc.dram_tensor("cl", [KD, P, NCODES], f16, kind="ExternalInput")
    b3_in = nc.dram_tensor("b3", [P, NCODES], f16, kind="ExternalInput")
    cb_in = nc.dram_tensor("cb", [NCODES, D], f32, kind="ExternalInput")
    iota_in = nc.dram_tensor("iota", [P, P], f32, kind="ExternalInput")
    offs_in = nc.dram_tensor("offs", [P, N_CH * K], f32, kind="ExternalInput")

    out_o = nc.dram_tensor("out", [TOK, D], f32, kind="ExternalOutput")
    ids_o = nc.dram_tensor("ids", [TOK, K], i32, kind="ExternalOutput")

    with tile.TileContext(nc) as tc:
        with (
            tc.tile_pool(name="const", bufs=1) as cpool,
            tc.tile_pool(name="cstream", bufs=3) as cspool,
            tc.tile_pool(name="sc", bufs=6) as spool,
            tc.tile_pool(name="merge", bufs=2) as mpool,
            tc.tile_pool(name="gath", bufs=2) as gpool,
            tc.tile_pool(name="outp", bufs=2) as opool,
            tc.tile_pool(name="psum", bufs=6, space="PSUM") as ppool,
        ):
            # --- resident constants ---
            xh = cpool.tile([P, KD * TOK], f16)
            xl = cpool.tile([P, KD * TOK], f16)
            for k in range(KD):
                nc.sync.dma_start(xh[:, k * TOK:(k + 1) * TOK], xh_in[k])
                nc.sync.dma_start(xl[:, k * TOK:(k + 1) * TOK], xl_in[k])
            b3 = cpool.tile([P, NCODES], f16)
            nc.sync.dma_start(b3[:], b3_in[:])
            ones3 = cpool.tile([P, P], f16)
            nc.vector.memset(ones3[:], 1.0)
            iota = cpool.tile([P, P], f32)
            nc.sync.dma_start(iota[:], iota_in[:])
            offs = cpool.tile([P, N_CH * K], f32)
            nc.sync.dma_start(offs[:], offs_in[:])
            # candidate values / local idx for all token tiles
            cv = cpool.tile([P, M_TILES * N_CH * K], f32)
            ci = cpool.tile([P, M_TILES * N_CH * K], u32)

            # --- score + chunk-local top-8, in groups of token tiles so the
            # merge/gather/mean of group g overlaps group g+1's matmuls ---
            GRP = 2
            inv_k = 1.0 / K

            def score_group(ms):
                for n in range(N_CH):
                    ns = slice(n * NB, (n + 1) * NB)
                    cht = cspool.tile([P, KD * NB], f16, tag="cht")
                    clt = cspool.tile([P, KD * NB], f16, tag="clt")
                    for k in range(KD):
                        nc.sync.dma_start(cht[:, k * NB:(k + 1) * NB], ch_in[k, :, ns])
                        nc.sync.dma_start(clt[:, k * NB:(k + 1) * NB], cl_in[k, :, ns])
                    for m in ms:
                        ps = ppool.tile([P, NB], f32)
                        nc.tensor.matmul(ps[:], lhsT=ones3[:], rhs=b3[:, ns],
                                         start=True, stop=False)
                        i = 0
                        for a, b in ((xh, cht), (xh, clt), (xl, cht)):
                            for k in range(KD):
                                nc.tensor.matmul(
                                    ps[:],
                                    lhsT=a[:, k * TOK + m * P: k * TOK + (m + 1) * P],
                                    rhs=b[:, k * NB:(k + 1) * NB],
                                    start=False, stop=(i == 3 * KD - 1),
                                )
                                i += 1
                        sc = spool.tile([P, NB], f32, tag="sc")
                        nc.scalar.copy(sc[:], ps[:])
                        cvs = cv[:, (m * N_CH + n) * K: (m * N_CH + n + 1) * K]
                        cis = ci[:, (m * N_CH + n) * K: (m * N_CH + n + 1) * K]
                        nc.vector.max(cvs, sc[:])
                        nc.vector.max_index(cis, cvs, sc[:])

            # --- merge + gather + mean for one token tile ---
            def finish_tile(m):
                cvm = cv[:, m * N_CH * K: (m + 1) * N_CH * K]
                cim = ci[:, m * N_CH * K: (m + 1) * N_CH * K]
                cif = mpool.tile([P, N_CH * K], f32, tag="cif")
                nc.vector.tensor_copy(cif[:], cim)
                gid = mpool.tile([P, N_CH * K], f32, tag="gid")
                nc.vector.tensor_add(gid[:], cif[:], offs[:])
                v8 = mpool.tile([P, K], f32, tag="v8")
                nc.vector.max(v8[:], cvm)
                pos8 = mpool.tile([P, K], u32, tag="pos8")
                nc.vector.max_index(pos8[:], v8[:], cvm)
                pos8f = mpool.tile([P, K], f32, tag="pos8f")
                nc.vector.tensor_copy(pos8f[:], pos8[:])
                idsf = mpool.tile([P, K], f32, tag="idsf")
                from concourse import mybir as mb
                for k in range(K):
                    sel = mpool.tile([P, N_CH * K], f32, tag="sel")
                    nc.vector.scalar_tensor_tensor(
                        sel[:], iota[:], pos8f[:, k:k + 1], gid[:],
                        op0=mb.AluOpType.is_equal, op1=mb.AluOpType.mult,
                    )
                    nc.vector.tensor_reduce(
                        idsf[:, k:k + 1], sel[:],
                        axis=mb.AxisListType.X, op=mb.AluOpType.add,
                    )
                ids32 = mpool.tile([P, K], i32, tag="ids32")
                nc.vector.tensor_copy(ids32[:], idsf[:])
                nc.sync.dma_start(ids_o[m * P:(m + 1) * P, :], ids32[:])

                import concourse.bass as bass
                g = gpool.tile([P, K, D], f32, tag="g")
                for k in range(K):
                    nc.gpsimd.indirect_dma_start(
                        out=g[:, k, :], out_offset=None,
                        in_=cb_in[:],
                        in_offset=bass.IndirectOffsetOnAxis(
                            ap=ids32[:, k:k + 1], axis=0),
                    )
                acc = opool.tile([P, D], f32, tag="acc")
                nc.vector.tensor_add(acc[:], g[:, 0, :], g[:, 1, :])
                for k in range(2, K):
                    nc.vector.tensor_add(acc[:], acc[:], g[:, k, :])
                outt = opool.tile([P, D], f32, tag="outt")
                nc.scalar.mul(outt[:], acc[:], inv_k)
                nc.sync.dma_start(out_o[m * P:(m + 1) * P, :], outt[:])

            for g in range(0, M_TILES, GRP):
                score_group(range(g, g + GRP))
                for m in range(g, g + GRP):
                    finish_tile(m)

    nc.compile()
    return nc


def _get_nc():
    if "nc" not in _CACHE:
        _CACHE["nc"] = _build()
    return _CACHE["nc"]


def _split_f16(a32):
    """3-term fp16 split helper: returns (hi, lo) with hi+lo ~ a32 (f32)."""
    hi = a32.astype(np.float16)
    lo = (a32 - hi.astype(np.float32)).astype(np.float16)
    return hi, lo


def _prep_in_maps(x, codebook):
    B, S, _ = x.shape
    xf = np.ascontiguousarray(x.reshape(B * S, D))
    cb = np.ascontiguousarray(codebook.astype(np.float32, copy=False))

    # codebook-side prep (shared by all cores)
    cT = cb.T.astype(np.float32)                      # [768, 8192]
    ch, cl = _split_f16(cT)
    ch = np.ascontiguousarray(ch.reshape(KD, P, NCODES))
    cl = np.ascontiguousarray(cl.reshape(KD, P, NCODES))
    bias = (-0.5 * (cb.astype(np.float64) ** 2).sum(1)).astype(np.float32)
    b0 = bias.astype(np.float16)
    r = bias - b0.astype(np.float32)
    b1 = r.astype(np.float16)
    b2 = (r - b1.astype(np.float32)).astype(np.float16)
    b3 = np.zeros((P, NCODES), np.float16)
    b3[0], b3[1], b3[2] = b0, b1, b2  # zero-padded to K=128 rows
    iota = np.broadcast_to(np.arange(N_CH * K, dtype=np.float32), (P, N_CH * K))
    iota = np.ascontiguousarray(iota)
    offs = np.broadcast_to(
        (np.arange(N_CH * K, dtype=np.float32) // K).astype(np.float32) * NB,
        (P, N_CH * K))
    offs = np.ascontiguousarray(offs)

    in_maps = []
    for c in range(N_CORES):
        xs = xf[c * TOK:(c + 1) * TOK]                # [1024, 768]
        xT = np.ascontiguousarray(xs.T)               # [768, 1024]
        xh, xl = _split_f16(xT)
        in_maps.append({
            "xh": np.ascontiguousarray(xh.reshape(KD, P, TOK)),
            "xl": np.ascontiguousarray(xl.reshape(KD, P, TOK)),
            "ch": ch, "cl": cl, "b3": b3, "cb": cb,
            "iota": iota, "offs": offs,
        })
    return in_maps


def kernel(x, codebook):
    from concourse.bass_utils import run_bass_kernel_spmd

    nc = _get_nc()
    B, S, _ = x.shape
    in_maps = _prep_in_maps(x, codebook)

    last_err = None
    for _attempt in range(3):
        try:
            res = run_bass_kernel_spmd(nc, in_maps, list(range(N_CORES)))
            break
        except Exception as e:  # device wedge — retry
            last_err = e
    else:
        raise last_err
    outs = np.concatenate([res.results[c]["out"] for c in range(N_CORES)])
    ids = np.concatenate([res.results[c]["ids"] for c in range(N_CORES)])
    return outs.reshape(B, S, D), ids.reshape(B, S, K).astype(np.int32)


# revision 15
# speedup vs baseline: 1.0381x; 1.0176x over previous
"""VQ codebook layer (EuclideanSnapFunction) on 8 Trainium2 NeuronCores.

Math: for each token x_t (768-d), scores over 8192 codes:
    s[t,n] = x_t . c_n - 0.5*||c_n||^2
which orders identically to -||x_t - c_n||^2 (the per-token ||x_t||^2 shift
does not change per-row order). Top-8 codes per token (descending score,
ties -> lower index, matching jax.lax.top_k), then outputs = mean of the 8
gathered codebook rows.

Sharding: data-parallel over tokens: 8192 tokens -> 8 cores x 1024 tokens,
codebook replicated.

Precision: scores are computed with a 3-term fp16 split matmul
    x.c ~= xh.ch + xh.cl + xl.ch   (xh=f16(x), xl=f16(x-xh), etc)
which carries ~21 significand bits through the fp32 PSUM accumulator;
measured |err| ~3e-5 vs fp64 — at the same level as a plain fp32 matmul —
while streaming at the full 1 cycle/row PE rate (fp32 matmul is 4 cyc/row).
The -0.5*||c||^2 bias is folded into the same PSUM accumulation as a K=3
fp16 matmul of a ones-vector against a 3-term fp16 split of the bias row.

Per core: 16 code-chunks of 512 stream through PSUM; per 128-token tile a
chunk-local top-8 (DVE max8/max8-index) produces 16x8 candidates; a final
max8 over the 128 candidate values + one-hot index extraction yields the
global top-8 ids. Codebook rows are gathered with indirect DMA and averaged.
"""
import numpy as np

try:
    import concourse  # noqa: F401
except ImportError:
    import sys
    sys.path.insert(0, "/opt/trn_rl_repo")

N_CORES = 8
P = 128
D = 768
KD = D // P            # 6 contraction chunks of 128
NCODES = 8192
TOK = 1024             # tokens per core
M_TILES = TOK // P     # 8 token tiles per core
NB = 512               # codes per PSUM chunk
N_CH = NCODES // NB    # 16 chunks
K = 8                  # top-k

_CACHE = {}


def _build():
    import concourse.tile as tile
    from concourse import bacc, mybir

    nc = bacc.Bacc("TRN2", target_bir_lowering=False, debug=False,
                   num_devices=N_CORES)
    f16 = mybir.dt.float16
    f32 = mybir.dt.float32
    u32 = mybir.dt.uint32
    i32 = mybir.dt.int32

    xh_in = nc.dram_tensor("xh", [KD, P, TOK], f16, kind="ExternalInput")
    xl_in = nc.dram_tensor("xl", [KD, P, TOK], f16, kind="ExternalInput")
    # chunk-major: [n, p, k*NB+j] so one contiguous DMA loads a whole chunk
    ch_in = nc.dram_tensor("ch", [N_CH, P, KD * NB], f16, kind="ExternalInput")
    cl_in = nc.dram_tensor("cl", [N_CH, P, KD * NB], f16, kind="ExternalInput")
    br_in = nc.dram_tensor("br", [P, NCODES], f32, kind="ExternalInput")
    cb_in = nc.dram_tensor("cb", [NCODES, D], f32, kind="ExternalInput")
    iota_in = nc.dram_tensor("iota", [P, P], f32, kind="ExternalInput")
    offs_in = nc.dram_tensor("offs", [P, N_CH * K], f32, kind="ExternalInput")

    out_o = nc.dram_tensor("out", [TOK, D], f32, kind="ExternalOutput")
    ids_o = nc.dram_tensor("ids", [TOK, K], i32, kind="ExternalOutput")

    with tile.TileContext(nc) as tc:
        with (
            tc.tile_pool(name="const", bufs=1) as cpool,
            tc.tile_pool(name="cstream", bufs=3) as cspool,
            tc.tile_pool(name="sc", bufs=6) as spool,
            tc.tile_pool(name="merge", bufs=2) as mpool,
            tc.tile_pool(name="gath", bufs=2) as gpool,
            tc.tile_pool(name="outp", bufs=2) as opool,
            tc.tile_pool(name="psum", bufs=6, space="PSUM") as ppool,
        ):
            # --- resident constants ---
            xh = cpool.tile([P, KD * TOK], f16)
            xl = cpool.tile([P, KD * TOK], f16)
            for k in range(KD):
                nc.sync.dma_start(xh[:, k * TOK:(k + 1) * TOK], xh_in[k])
                nc.sync.dma_start(xl[:, k * TOK:(k + 1) * TOK], xl_in[k])
            br = cpool.tile([P, NCODES], f32)
            nc.sync.dma_start(br[:], br_in[:])
            iota = cpool.tile([P, P], f32)
            nc.sync.dma_start(iota[:], iota_in[:])
            offs = cpool.tile([P, N_CH * K], f32)
            nc.sync.dma_start(offs[:], offs_in[:])
            # candidate values / local idx for all token tiles
            cv = cpool.tile([P, M_TILES * N_CH * K], f32)
            ci = cpool.tile([P, M_TILES * N_CH * K], u32)

            # --- score + chunk-local top-8, in groups of token tiles so the
            # merge/gather/mean of group g overlaps group g+1's matmuls ---
            inv_k = 1.0 / K

            def score_group(ms):
                for n in range(N_CH):
                    ns = slice(n * NB, (n + 1) * NB)
                    cht = cspool.tile([P, KD * NB], f16, tag="cht")
                    clt = cspool.tile([P, KD * NB], f16, tag="clt")
                    nc.sync.dma_start(cht[:], ch_in[n])
                    nc.sync.dma_start(clt[:], cl_in[n])
                    for m in ms:
                        ps = ppool.tile([P, NB], f32)
                        i = 0
                        for a, b in ((xh, cht), (xh, clt), (xl, cht)):
                            for k in range(KD):
                                nc.tensor.matmul(
                                    ps[:],
                                    lhsT=a[:, k * TOK + m * P: k * TOK + (m + 1) * P],
                                    rhs=b[:, k * NB:(k + 1) * NB],
                                    start=(i == 0), stop=(i == 3 * KD - 1),
                                )
                                i += 1
                        sc = spool.tile([P, NB], f32, tag="sc")
                        nc.vector.tensor_add(sc[:], ps[:], br[:, ns])
                        cvs = cv[:, (m * N_CH + n) * K: (m * N_CH + n + 1) * K]
                        cis = ci[:, (m * N_CH + n) * K: (m * N_CH + n + 1) * K]
                        nc.vector.max(cvs, sc[:])
                        nc.vector.max_index(cis, cvs, sc[:])

            # --- merge + gather + mean for one token tile ---
            def finish_tile(m):
                cvm = cv[:, m * N_CH * K: (m + 1) * N_CH * K]
                cim = ci[:, m * N_CH * K: (m + 1) * N_CH * K]
                cif = mpool.tile([P, N_CH * K], f32, tag="cif")
                nc.vector.tensor_copy(cif[:], cim)
                gid = mpool.tile([P, N_CH * K], f32, tag="gid")
                nc.vector.tensor_add(gid[:], cif[:], offs[:])
                v8 = mpool.tile([P, K], f32, tag="v8")
                nc.vector.max(v8[:], cvm)
                pos8 = mpool.tile([P, K], u32, tag="pos8")
                nc.vector.max_index(pos8[:], v8[:], cvm)
                pos8f = mpool.tile([P, K], f32, tag="pos8f")
                nc.vector.tensor_copy(pos8f[:], pos8[:])
                idsf = mpool.tile([P, K], f32, tag="idsf")
                from concourse import mybir as mb
                for k in range(K):
                    sel = mpool.tile([P, N_CH * K], f32, tag="sel")
                    nc.vector.scalar_tensor_tensor(
                        sel[:], iota[:], pos8f[:, k:k + 1], gid[:],
                        op0=mb.AluOpType.is_equal, op1=mb.AluOpType.mult,
                    )
                    nc.vector.tensor_reduce(
                        idsf[:, k:k + 1], sel[:],
                        axis=mb.AxisListType.X, op=mb.AluOpType.add,
                    )
                ids32 = mpool.tile([P, K], i32, tag="ids32")
                nc.vector.tensor_copy(ids32[:], idsf[:])
                nc.sync.dma_start(ids_o[m * P:(m + 1) * P, :], ids32[:])

                import concourse.bass as bass
                g = gpool.tile([P, K, D], f32, tag="g")
                for k in range(K):
                    nc.gpsimd.indirect_dma_start(
                        out=g[:, k, :], out_offset=None,
                        in_=cb_in[:],
                        in_offset=bass.IndirectOffsetOnAxis(
                            ap=ids32[:, k:k + 1], axis=0),
                    )
                acc = opool.tile([P, D], f32, tag="acc")
                nc.vector.tensor_add(acc[:], g[:, 0, :], g[:, 1, :])
                for k in range(2, K):
                    nc.vector.tensor_add(acc[:], acc[:], g[:, k, :])
                outt = opool.tile([P, D], f32, tag="outt")
                nc.scalar.mul(outt[:], acc[:], inv_k)
                nc.sync.dma_start(out_o[m * P:(m + 1) * P, :], outt[:])

            # groups of 2, with the last two tiles as singleton groups so the
            # final exposed merge/gather tail is one token-tile deep
            groups = [[0, 1], [2, 3], [4, 5], [6], [7]]
            for ms in groups:
                score_group(ms)
                for m in ms:
                    finish_tile(m)

    nc.compile()
    return nc


def _get_nc():
    if "nc" not in _CACHE:
        _CACHE["nc"] = _build()
    return _CACHE["nc"]


def _split_f16(a32):
    """3-term fp16 split helper: returns (hi, lo) with hi+lo ~ a32 (f32)."""
    hi = a32.astype(np.float16)
    lo = (a32 - hi.astype(np.float32)).astype(np.float16)
    return hi, lo


def _prep_in_maps(x, codebook):
    B, S, _ = x.shape
    xf = np.ascontiguousarray(x.reshape(B * S, D))
    cb = np.ascontiguousarray(codebook.astype(np.float32, copy=False))

    # codebook-side prep (shared by all cores)
    cT = cb.T.astype(np.float32)                      # [768, 8192]
    ch, cl = _split_f16(cT)

    def _chunk_major(a):
        # [KD*P, NCODES] -> [N_CH, P, KD*NB]: chunk-contiguous for 1-DMA loads
        a4 = a.reshape(KD, P, N_CH, NB).transpose(2, 1, 0, 3)
        return np.ascontiguousarray(a4.reshape(N_CH, P, KD * NB))

    ch = _chunk_major(ch)
    cl = _chunk_major(cl)
    bias = (-0.5 * (cb.astype(np.float64) ** 2).sum(1)).astype(np.float32)
    br = np.ascontiguousarray(np.broadcast_to(bias, (P, NCODES)))
    iota = np.broadcast_to(np.arange(N_CH * K, dtype=np.float32), (P, N_CH * K))
    iota = np.ascontiguousarray(iota)
    offs = np.broadcast_to(
        (np.arange(N_CH * K, dtype=np.float32) // K).astype(np.float32) * NB,
        (P, N_CH * K))
    offs = np.ascontiguousarray(offs)

    in_maps = []
    for c in range(N_CORES):
        xs = xf[c * TOK:(c + 1) * TOK]                # [1024, 768]
        xT = np.ascontiguousarray(xs.T)               # [768, 1024]
        xh, xl = _split_f16(xT)
        in_maps.append({
            "xh": np.ascontiguousarray(xh.reshape(KD, P, TOK)),
            "xl": np.ascontiguousarray(xl.reshape(KD, P, TOK)),
            "ch": ch, "cl": cl, "br": br, "cb": cb,
            "iota": iota, "offs": offs,
        })
    return in_maps


def kernel(x, codebook):
    from concourse.bass_utils import run_bass_kernel_spmd

    nc = _get_nc()
    B, S, _ = x.shape
    in_maps = _prep_in_maps(x, codebook)

    last_err = None
    for _attempt in range(3):
        try:
            res = run_bass_kernel_spmd(nc, in_maps, list(range(N_CORES)))
            break
        except Exception as e:  # device wedge — retry
            last_err = e
    else:
        raise last_err
    outs = np.concatenate([res.results[c]["out"] for c in range(N_CORES)])
    ids = np.concatenate([res.results[c]["ids"] for c in range(N_CORES)])
    return outs.reshape(B, S, D), ids.reshape(B, S, K).astype(np.int32)
